# revision 12
# baseline (speedup 1.0000x reference)
"""Self-contained 8-core Trainium2 Bass kernel for MultiHeadAttention.

Problem: B=2, S=2048, D=1024, H=16 heads (hd=64), f32, self-attention
(no mask), eval mode (dropout = identity).

Sharding: data-parallel over B (2) x tensor-parallel over heads (4 groups
of 4 heads) = 8 cores. Each core computes, for its batch b and its 4
heads: Q/K/V projections (column-sliced), attention, and a partial
output projection (row-sliced Wo). Host sums the 4 partials per batch
and adds the (bv @ Wo + bo) correction (bv never enters the kernel:
ctx rows sum probs to 1, so (ctx+bv) @ Wo = ctx @ Wo + bv @ Wo).

Algebraic simplifications used (exact):
  - bk dropped: softmax over k is invariant to the per-q constant Q.bk.
  - softmax computed without max subtraction (scores bounded ~|s|<10).
  - bq folded into Q^T as a per-partition bias (per-q constant cancels
    in softmax).
  - row normalization deferred past the P@V matmul (scale ctx instead
    of probs); row sums obtained free via an appended ones-column in V.

v3 design (ACT-bound pipeline):
  - All inputs bf16, host-prearranged so every DMA is contiguous per
    partition; xt arrives in 4 column-group DMAs so compute starts
    after ~1MB.
  - scores^T per head pair via two tile_position row-group matmuls
    (K=64 each) running concurrently on the PE.
  - exp on ACT is the critical engine (16.8M elems/core at 1
    elem/cycle/lane ~= 110us); everything else hides behind it.
  - PV: ctx^T[65, q] += [V_h | 1]^T @ exp^T accumulated over k tiles;
    row 64 is the softmax denominator (free).
  - normalization decoupled from PSUM: ctx+denom copied to SBUF
    (frees the bank in ~0.6us), then reciprocal_approx_fast +
    partition_broadcast + mul off the critical path.
  - All projection/out-projection matmuls interleaved into the
    ACT-bound attention r-loops via generators with deadline-aware
    ordering; feed slots run before the score matmuls of each r
    iteration so a fed op is never queued behind its consumer on the
    PE FIFO.
"""

import sys

sys.path.insert(0, "/opt/trn_rl_repo")

import numpy as np

B, S, D, H, HD = 2, 2048, 1024, 16, 64
HPC = 4  # heads per core
NCORES = 8
DC = D // 128  # 8 contraction chunks
ST = S // 128  # 16 s-tiles
QCW = 512  # q chunk width
QC = S // QCW  # 4 q chunks
KT = S // 128  # 16 k tiles

_CACHE = {}


def _build(repeat=1):
    import concourse.bass as bass  # noqa: F401
    import concourse.mybir as mybir
    import concourse.tile as tile
    from concourse import bacc
    from concourse.library_config import attn as attn_lib

    F32 = mybir.dt.float32
    BF16 = mybir.dt.bfloat16
    AF = mybir.ActivationFunctionType

    nc = bacc.Bacc("TRN2", target_bir_lowering=False, debug=False)

    # host-prearranged layouts (all contiguous per partition):
    #   xt  [128, QC, DC, 512] : xt[p, g, c, s] = x[c*128+p, g*512+s]
    #   wq/wk/wv [128, DC, 256]: w[p, c, n] = W[c*128+p, n]
    #   wo  [128, 2, 1024]     : wo[p, e, n] = Wo[e*128+p, n]
    xt_d = nc.dram_tensor("xt", [128, QC, DC, QCW], BF16, kind="ExternalInput")
    wq_d = nc.dram_tensor("wq", [128, DC, HPC * HD], BF16, kind="ExternalInput")
    wk_d = nc.dram_tensor("wk", [128, DC, HPC * HD], BF16, kind="ExternalInput")
    wv_d = nc.dram_tensor("wv", [128, DC, HPC * HD], BF16, kind="ExternalInput")
    wo_d = nc.dram_tensor("wo", [128, 2, D], BF16, kind="ExternalInput")
    bq_d = nc.dram_tensor("bq2", [128, 2], F32, kind="ExternalInput")
    out_d = nc.dram_tensor("out_p", [S, D], BF16, kind="ExternalOutput")

    with tile.TileContext(nc) as tc:
        nc.gpsimd.load_library(attn_lib)
        with (
            tc.tile_pool(name="wp", bufs=1) as wp,
            tc.tile_pool(name="xp", bufs=1) as xp,
            tc.tile_pool(name="qk", bufs=1) as qk,
            tc.tile_pool(name="vp", bufs=1) as vp,
            tc.tile_pool(name="ep", bufs=3) as ep,
            tc.tile_pool(name="cp", bufs=1) as cp,
            tc.tile_pool(name="cu", bufs=4) as cu,
            tc.tile_pool(name="mp", bufs=4) as mp,
            tc.tile_pool(name="op", bufs=2) as op,
            tc.tile_pool(name="pp", bufs=2, space="PSUM") as pp,
        ):
            # ---- loads split across both HWDGE queues (SP + Act), ordered
            # by first use: V needs wv+g0, kt needs wk+g0, qt needs wq+bq.
            xt_t = xp.tile([128, QC, DC, QCW], BF16, tag="xt")
            nc.scalar.dma_start(xt_t[:, 0, 0:4], xt_d[:, 0, 0:4])
            wq_t = wp.tile([128, DC, HPC * HD], BF16, tag="wq")
            nc.scalar.dma_start(wq_t[:], wq_d[:])
            nc.scalar.dma_start(xt_t[:, 2], xt_d[:, 2])
            wv_t = wp.tile([128, DC, HPC * HD], BF16, tag="wv")
            nc.sync.dma_start(wv_t[:], wv_d[:])
            wk_t = wp.tile([128, DC, HPC * HD], BF16, tag="wk")
            nc.sync.dma_start(wk_t[:], wk_d[:])
            nc.sync.dma_start(xt_t[:, 0, 4:8], xt_d[:, 0, 4:8])
            bq_t = wp.tile([128, 2], F32, tag="bq")
            nc.sync.dma_start(bq_t[:], bq_d[:])
            nc.sync.dma_start(xt_t[:, 1], xt_d[:, 1])
            nc.sync.dma_start(xt_t[:, 3], xt_d[:, 3])
            wo_t = wp.tile([128, 2, D], BF16, tag="wo")
            nc.sync.dma_start(wo_t[:], wo_d[:])
            ones_f = wp.tile([128, 64], BF16, tag="onesf")
            nc.vector.memset(ones_f[:], 1.0)
            # warm the ACT exp table during the DMA prefix
            warm_in = wp.tile([1, 2], F32, tag="warm_i")
            nc.vector.memset(warm_in[:], 0.0)
            warm_out = wp.tile([1, 2], F32, tag="warm_o")
            nc.scalar.activation(warm_out[:], warm_in[:], AF.Exp)
            # warm the PE (HAM un-throttle needs ~3.4us of sustained matmuls)
            wps = pp.tile([64, 64], F32, tag="vo", bufs=1, name="wps")
            for i in range(110):
                nc.tensor.matmul(wps[:], ones_f[:, 0:64], ones_f[:, 0:64],
                                 start=(i == 0), stop=(i == 109))

            import contextlib
            if repeat > 1:
                _engs = [mybir.EngineType.PE, mybir.EngineType.Activation,
                         mybir.EngineType.DVE, mybir.EngineType.SP,
                         mybir.EngineType.Pool]
                rep_ctx = tc.For_i(0, repeat, hint_engines=_engs, staggered_reset=True)
            else:
                rep_ctx = contextlib.nullcontext()
            with rep_ctx:
                # ---- V projection -> v1 [s, 4*(64+1)] with ones columns
                v1_t = vp.tile([128, ST, HPC * 65], BF16, tag="v1")
                with nc.allow_low_precision(reason="bf16 matmul operands"):
                    nc.vector.tensor_copy(
                        v1_t[:].rearrange("p s (h c) -> p s h c", c=65)[:, :, :, 64],
                        ones_f[:, 0:64].rearrange("p (s h) -> p s h", s=ST),
                    )

                def v_proj(st):
                    g, off = st // 4, (st % 4) * 128
                    vps = pp.tile([128, HPC * HD], F32, tag="vo", bufs=1, name="vps")
                    for c in range(DC):
                        nc.tensor.matmul(
                            vps[:],
                            xt_t[:, g, c, off:off + 128],
                            wv_t[:, c, :],
                            start=(c == 0),
                            stop=(c == DC - 1),
                        )
                    with nc.allow_low_precision(reason="bf16 matmul operands"):
                        nc.vector.tensor_copy(
                            v1_t[:, st, :].rearrange("p (h c) -> p h c", c=65)[:, :, 0:64],
                            vps[:].rearrange("p (h c) -> p h c", c=64),
                        )

                # ---- Q^T / K^T projections (per head pair, bf16)
                qt_tiles = [qk.tile([128, S], BF16, tag=f"qt{p}", name=f"qt{p}") for p in range(2)]
                kt_tiles = [qk.tile([128, S], BF16, tag=f"kt{p}", name=f"kt{p}") for p in range(2)]

                def kt_proj(pair, qc):
                    qs = slice(qc * QCW, (qc + 1) * QCW)
                    kps = pp.tile([128, QCW], F32, tag="qkv", bufs=1, name="kps")
                    for c in range(DC):
                        nc.tensor.matmul(
                            kps[:],
                            wk_t[:, c, pair * 128:(pair + 1) * 128],
                            xt_t[:, qc, c, :],
                            start=(c == 0),
                            stop=(c == DC - 1),
                        )
                        yield
                    with nc.allow_low_precision(reason="bf16 matmul operands"):
                        nc.vector.tensor_copy(kt_tiles[pair][:, qs], kps[:])
                    yield

                def qt_proj(pair, qc):
                    qs = slice(qc * QCW, (qc + 1) * QCW)
                    qps = pp.tile([128, QCW], F32, tag="qkv", bufs=1, name="qps")
                    for c in range(DC):
                        nc.tensor.matmul(
                            qps[:],
                            wq_t[:, c, pair * 128:(pair + 1) * 128],
                            xt_t[:, qc, c, :],
                            start=(c == 0),
                            stop=(c == DC - 1),
                        )
                        yield
                    with nc.allow_low_precision(reason="bf16 score operands"):
                        nc.vector.tensor_scalar_add(
                            qt_tiles[pair][:, qs], qps[:], bq_t[:, pair:pair + 1]
                        )
                    yield

                ctxt_tiles = [cp.tile([128, S], BF16, tag=f"ct{p}", name=f"ct{p}") for p in range(2)]

                def attention(pair, qc, feed=None, slots=2, feed_start_r=0, inline_v=False,
                              normalize_now=False):
                    qs = slice(qc * QCW, (qc + 1) * QCW)
                    ctx_ps = [pp.tile([65, QCW], F32, tag="ctx", name=f"ctx{_h}", bufs=2) for _h in range(2)]
                    for r in range(KT):
                        if feed is not None and r >= feed_start_r:
                            for _ in range(slots):
                                next(feed, None)
                        sreg = pp.tile([128, 2 * QCW], F32, tag="big")
                        expt = ep.tile([128, 2 * QCW], BF16, tag="exp")
                        for h in range(2):
                            nc.tensor.matmul(
                                sreg[:, h * QCW:(h + 1) * QCW],
                                kt_tiles[pair][64 * h:64 * (h + 1), r * 128:(r + 1) * 128],
                                qt_tiles[pair][64 * h:64 * (h + 1), qs],
                                start=True,
                                stop=True,
                                tile_position=(64 * h, 0),
                            )
                        with nc.allow_low_precision(reason="bf16 exp output"):
                            nc.scalar.activation(expt[:], sreg[:], AF.Exp, scale=0.125)
                        if inline_v and r <= 13:
                            v_proj(r + 2)
                        for h in range(2):
                            hh = 2 * pair + h
                            nc.tensor.matmul(
                                ctx_ps[h][:],
                                v1_t[:, r, 65 * hh:65 * hh + 65],
                                expt[:, h * QCW:(h + 1) * QCW],
                                start=(r == 0),
                                stop=(r == KT - 1),
                            )
                    if normalize_now:
                        # final call: nothing reuses the ctx banks, so
                        # normalize straight from PSUM (skips the SBUF
                        # staging copies on the tail critical path).
                        rds = []
                        for h in range(2):
                            dcp = mp.tile([1, QCW], F32, tag="dcp", name=f"dcp{h}")
                            nc.vector.tensor_copy(dcp[:], ctx_ps[h][64:65, :])
                            rd = mp.tile([1, QCW], F32, tag="rd", name=f"rd{h}")
                            nc.vector.reciprocal_approx_fast(rd[:], dcp[:])
                            rds.append(rd)
                        bcts = []
                        for h in range(2):
                            bct = mp.tile([64, QCW], F32, tag="bc", name=f"bct{h}")
                            nc.gpsimd.partition_broadcast(bct[:], rds[h][:])
                            bcts.append(bct)
                        for h in range(2):
                            with nc.allow_low_precision(reason="bf16 ctx"):
                                nc.vector.tensor_mul(
                                    ctxt_tiles[pair][64 * h:64 * (h + 1), qs],
                                    ctx_ps[h][0:64, :],
                                    bcts[h][:],
                                )
                        return iter(())
                    # evacuate PSUM now (frees ctx banks for the next call);
                    # the rest of the normalization is returned as a
                    # generator for injection into the next call's feed.
                    ctxus = []
                    for h in range(2):
                        ctxu = cu.tile([65, QCW], F32, tag="ctxu", name=f"ctxu{h}")
                        nc.vector.tensor_copy(ctxu[:], ctx_ps[h][:])
                        ctxus.append(ctxu)

                    def _norm_tail():
                        rds = []
                        for h in range(2):
                            dcp = mp.tile([1, QCW], F32, tag="dcp", name=f"dcp{h}")
                            nc.vector.tensor_copy(dcp[:], ctxus[h][64:65, :])
                            yield
                            rd = mp.tile([1, QCW], F32, tag="rd", name=f"rd{h}")
                            nc.vector.reciprocal_approx_fast(rd[:], dcp[:])
                            rds.append(rd)
                            yield
                        bcts = []
                        for h in range(2):
                            bct = mp.tile([64, QCW], F32, tag="bc", name=f"bct{h}")
                            nc.gpsimd.partition_broadcast(bct[:], rds[h][:])
                            bcts.append(bct)
                            yield
                        for h in range(2):
                            with nc.allow_low_precision(reason="bf16 ctx"):
                                nc.vector.tensor_mul(
                                    ctxt_tiles[pair][64 * h:64 * (h + 1), qs],
                                    ctxus[h][0:64, :],
                                    bcts[h][:],
                                )
                            yield

                    return _norm_tail()

                def outproj_sub(qc, sub):
                    q0 = qc * QCW + sub * 128
                    osb = op.tile([128, D], BF16, tag="osb")
                    for d2 in range(2):
                        ops = pp.tile([128, 512], F32, tag=("vo" if d2 == 0 else "qkv"), bufs=1)
                        for pair in range(2):
                            nc.tensor.matmul(
                                ops[:],
                                ctxt_tiles[pair][:, q0:q0 + 128],
                                wo_t[:, pair, d2 * 512:(d2 + 1) * 512],
                                start=(pair == 0),
                                stop=(pair == 1),
                            )
                            yield
                        with nc.allow_low_precision(reason="bf16 out"):
                            nc.vector.tensor_copy(osb[:, d2 * 512:(d2 + 1) * 512], ops[:])
                        yield
                    nc.sync.dma_start(out_d[q0:q0 + 128, :], osb[:])
                    yield

                def chain(*gens):
                    for g in gens:
                        yield from g

                def drain(g):
                    for _ in g:
                        pass

                # ---- schedule: minimal prefix, then ACT-bound attention
                # with everything else interleaved.
                drain(kt_proj(0, 0))
                drain(qt_proj(0, 0))
                for st in range(2):
                    v_proj(st)

                feed = chain(kt_proj(0, 1), kt_proj(0, 2), kt_proj(0, 3), qt_proj(0, 1))
                nt = attention(0, 0, feed, slots=3, feed_start_r=1, inline_v=True)
                drain(feed)
                feed = chain(nt, kt_proj(1, 0), kt_proj(1, 1), qt_proj(0, 2))
                nt = attention(0, 1, feed)
                drain(feed)
                feed = chain(nt, kt_proj(1, 2), kt_proj(1, 3), qt_proj(0, 3))
                nt = attention(0, 2, feed)
                drain(feed)
                feed = chain(nt, qt_proj(1, 0), qt_proj(1, 1))
                nt = attention(0, 3, feed)
                drain(feed)
                feed = chain(nt, qt_proj(1, 2))
                nt = attention(1, 0, feed)
                drain(feed)
                feed = chain(nt, qt_proj(1, 3), outproj_sub(0, 0), outproj_sub(0, 1))
                nt = attention(1, 1, feed)
                drain(feed)
                feed = chain(nt, outproj_sub(0, 2), outproj_sub(0, 3),
                             outproj_sub(1, 0), outproj_sub(1, 1))
                nt = attention(1, 2, feed)
                drain(feed)
                feed = chain(nt, outproj_sub(1, 2), outproj_sub(1, 3),
                             outproj_sub(2, 0), outproj_sub(2, 1))
                nt = attention(1, 3, feed, normalize_now=True)
                drain(feed)
                drain(nt)
                drain(outproj_sub(2, 2))
                drain(outproj_sub(2, 3))
                for sub in range(4):
                    drain(outproj_sub(3, sub))

    nc.compile()
    return nc


def _get_nc(repeat=1):
    key = (repeat,)
    if key not in _CACHE:
        _CACHE[key] = _build(repeat)
    return _CACHE[key]


def _bf16(a):
    import ml_dtypes

    return np.ascontiguousarray(np.asarray(a, np.float32)).astype(ml_dtypes.bfloat16)


def _make_in_maps(query_input, Wq, bq, Wk, Wv, Wo):
    x = np.asarray(query_input, dtype=np.float32)
    in_maps = []
    for core in range(NCORES):
        b, g = divmod(core, NCORES // B)
        cs = slice(g * HPC * HD, (g + 1) * HPC * HD)
        # xt[p, g, c, s] = x[b][g*512+s, c*128+p]
        xr = x[b].reshape(QC, QCW, DC, 128).transpose(3, 0, 2, 1)
        in_maps.append({
            "xt": _bf16(xr),
            "wq": _bf16(Wq[:, cs].reshape(DC, 128, HPC * HD).transpose(1, 0, 2)),
            "wk": _bf16(Wk[:, cs].reshape(DC, 128, HPC * HD).transpose(1, 0, 2)),
            "wv": _bf16(Wv[:, cs].reshape(DC, 128, HPC * HD).transpose(1, 0, 2)),
            "wo": _bf16(Wo[cs, :].reshape(2, 128, D).transpose(1, 0, 2)),
            "bq2": np.ascontiguousarray(np.asarray(bq, np.float32)[cs].reshape(2, 128).T),
        })
    return in_maps


def kernel(query_input, Wq, bq, Wk, bk, Wv, bv, Wo, bo):
    from concourse.bass_utils import run_bass_kernel_spmd

    Wq = np.asarray(Wq, np.float32)
    Wk = np.asarray(Wk, np.float32)
    Wv = np.asarray(Wv, np.float32)
    Wo = np.asarray(Wo, np.float32)
    bq = np.asarray(bq, np.float32)
    bv = np.asarray(bv, np.float32)
    bo = np.asarray(bo, np.float32)

    nc = _get_nc()
    in_maps = _make_in_maps(query_input, Wq, bq, Wk, Wv, Wo)
    res = run_bass_kernel_spmd(nc, in_maps, core_ids=list(range(NCORES)))

    gpc = NCORES // B  # groups per batch
    out = np.zeros((B, S, D), np.float32)
    for core in range(NCORES):
        b = core // gpc
        out[b] += res.results[core]["out_p"].astype(np.float32)
    # bv correction (exact) + bo, applied once on the full output
    out += (bv @ Wo + bo)[None, None, :]
    return out


# revision 13
# speedup vs baseline: 1.0012x; 1.0012x over previous
"""Self-contained 8-core Trainium2 Bass kernel for MultiHeadAttention.

Problem: B=2, S=2048, D=1024, H=16 heads (hd=64), f32, self-attention
(no mask), eval mode (dropout = identity).

Sharding: data-parallel over B (2) x tensor-parallel over heads (4 groups
of 4 heads) = 8 cores. Each core computes, for its batch b and its 4
heads: Q/K/V projections (column-sliced), attention, and a partial
output projection (row-sliced Wo). Host sums the 4 partials per batch
and adds the (bv @ Wo + bo) correction (bv never enters the kernel:
ctx rows sum probs to 1, so (ctx+bv) @ Wo = ctx @ Wo + bv @ Wo).

Algebraic simplifications used (exact):
  - bk dropped: softmax over k is invariant to the per-q constant Q.bk.
  - softmax computed without max subtraction (scores bounded ~|s|<10).
  - bq folded into Q^T as a per-partition bias (per-q constant cancels
    in softmax).
  - row normalization deferred past the P@V matmul (scale ctx instead
    of probs); row sums obtained free via an appended ones-column in V.

v3 design (ACT-bound pipeline):
  - All inputs bf16, host-prearranged so every DMA is contiguous per
    partition; xt arrives in 4 column-group DMAs so compute starts
    after ~1MB.
  - scores^T per head pair via two tile_position row-group matmuls
    (K=64 each) running concurrently on the PE.
  - exp on ACT is the critical engine (16.8M elems/core at 1
    elem/cycle/lane ~= 110us); everything else hides behind it.
  - PV: ctx^T[65, q] += [V_h | 1]^T @ exp^T accumulated over k tiles;
    row 64 is the softmax denominator (free).
  - normalization decoupled from PSUM: ctx+denom copied to SBUF
    (frees the bank in ~0.6us), then reciprocal_approx_fast +
    partition_broadcast + mul off the critical path.
  - All projection/out-projection matmuls interleaved into the
    ACT-bound attention r-loops via generators with deadline-aware
    ordering; feed slots run before the score matmuls of each r
    iteration so a fed op is never queued behind its consumer on the
    PE FIFO.
"""

import sys

sys.path.insert(0, "/opt/trn_rl_repo")

import numpy as np

B, S, D, H, HD = 2, 2048, 1024, 16, 64
HPC = 4  # heads per core
NCORES = 8
DC = D // 128  # 8 contraction chunks
ST = S // 128  # 16 s-tiles
QCW = 512  # q chunk width
QC = S // QCW  # 4 q chunks
KT = S // 128  # 16 k tiles

_CACHE = {}


def _build(repeat=1):
    import concourse.bass as bass  # noqa: F401
    import concourse.mybir as mybir
    import concourse.tile as tile
    from concourse import bacc
    from concourse.library_config import attn as attn_lib

    F32 = mybir.dt.float32
    BF16 = mybir.dt.bfloat16
    AF = mybir.ActivationFunctionType

    nc = bacc.Bacc("TRN2", target_bir_lowering=False, debug=False)

    # host-prearranged layouts (all contiguous per partition):
    #   xt  [128, QC, DC, 512] : xt[p, g, c, s] = x[c*128+p, g*512+s]
    #   wq/wk/wv [128, DC, 256]: w[p, c, n] = W[c*128+p, n]
    #   wo  [128, 2, 1024]     : wo[p, e, n] = Wo[e*128+p, n]
    xt_d = nc.dram_tensor("xt", [128, QC, DC, QCW], BF16, kind="ExternalInput")
    wq_d = nc.dram_tensor("wq", [128, DC, HPC * HD], BF16, kind="ExternalInput")
    wk_d = nc.dram_tensor("wk", [128, DC, HPC * HD], BF16, kind="ExternalInput")
    wv_d = nc.dram_tensor("wv", [128, DC, HPC * HD], BF16, kind="ExternalInput")
    wo_d = nc.dram_tensor("wo", [128, 2, D], BF16, kind="ExternalInput")
    bq_d = nc.dram_tensor("bq2", [128, 2], F32, kind="ExternalInput")
    out_d = nc.dram_tensor("out_p", [S, D], BF16, kind="ExternalOutput")

    with tile.TileContext(nc) as tc:
        nc.gpsimd.load_library(attn_lib)
        with (
            tc.tile_pool(name="wp", bufs=1) as wp,
            tc.tile_pool(name="xp", bufs=1) as xp,
            tc.tile_pool(name="qk", bufs=1) as qk,
            tc.tile_pool(name="vp", bufs=1) as vp,
            tc.tile_pool(name="ep", bufs=3) as ep,
            tc.tile_pool(name="cp", bufs=1) as cp,
            tc.tile_pool(name="cu", bufs=4) as cu,
            tc.tile_pool(name="mp", bufs=4) as mp,
            tc.tile_pool(name="op", bufs=2) as op,
            tc.tile_pool(name="pp", bufs=2, space="PSUM") as pp,
        ):
            # ---- loads split across both HWDGE queues (SP + Act), ordered
            # by first use: V needs wv+g0, kt needs wk+g0, qt needs wq+bq.
            xt_t = xp.tile([128, QC, DC, QCW], BF16, tag="xt")
            nc.scalar.dma_start(xt_t[:, 0, 0:4], xt_d[:, 0, 0:4])
            wq_t = wp.tile([128, DC, HPC * HD], BF16, tag="wq")
            nc.scalar.dma_start(wq_t[:], wq_d[:])
            nc.scalar.dma_start(xt_t[:, 2], xt_d[:, 2])
            wk_t = wp.tile([128, DC, HPC * HD], BF16, tag="wk")
            nc.sync.dma_start(wk_t[:], wk_d[:])
            nc.sync.dma_start(xt_t[:, 0, 4:8], xt_d[:, 0, 4:8])
            wv_t = wp.tile([128, DC, HPC * HD], BF16, tag="wv")
            nc.sync.dma_start(wv_t[:], wv_d[:])
            bq_t = wp.tile([128, 2], F32, tag="bq")
            nc.sync.dma_start(bq_t[:], bq_d[:])
            nc.sync.dma_start(xt_t[:, 1], xt_d[:, 1])
            nc.sync.dma_start(xt_t[:, 3], xt_d[:, 3])
            wo_t = wp.tile([128, 2, D], BF16, tag="wo")
            nc.sync.dma_start(wo_t[:], wo_d[:])
            ones_f = wp.tile([128, 64], BF16, tag="onesf")
            nc.vector.memset(ones_f[:], 1.0)
            # warm the ACT exp table during the DMA prefix
            warm_in = wp.tile([1, 2], F32, tag="warm_i")
            nc.vector.memset(warm_in[:], 0.0)
            warm_out = wp.tile([1, 2], F32, tag="warm_o")
            nc.scalar.activation(warm_out[:], warm_in[:], AF.Exp)
            # warm the PE (HAM un-throttle needs ~3.4us of sustained matmuls)
            wps = pp.tile([64, 64], F32, tag="vo", bufs=1, name="wps")
            for i in range(150):
                nc.tensor.matmul(wps[:], ones_f[:, 0:64], ones_f[:, 0:64],
                                 start=(i == 0), stop=(i == 149))

            import contextlib
            if repeat > 1:
                _engs = [mybir.EngineType.PE, mybir.EngineType.Activation,
                         mybir.EngineType.DVE, mybir.EngineType.SP,
                         mybir.EngineType.Pool]
                rep_ctx = tc.For_i(0, repeat, hint_engines=_engs, staggered_reset=True)
            else:
                rep_ctx = contextlib.nullcontext()
            with rep_ctx:
                # ---- V projection -> v1 [s, 4*(64+1)] with ones columns
                v1_t = vp.tile([128, ST, HPC * 65], BF16, tag="v1")
                with nc.allow_low_precision(reason="bf16 matmul operands"):
                    nc.vector.tensor_copy(
                        v1_t[:].rearrange("p s (h c) -> p s h c", c=65)[:, :, :, 64],
                        ones_f[:, 0:64].rearrange("p (s h) -> p s h", s=ST),
                    )

                def v_proj(st):
                    g, off = st // 4, (st % 4) * 128
                    vps = pp.tile([128, HPC * HD], F32, tag="vo", bufs=1, name="vps")
                    for c in range(DC):
                        nc.tensor.matmul(
                            vps[:],
                            xt_t[:, g, c, off:off + 128],
                            wv_t[:, c, :],
                            start=(c == 0),
                            stop=(c == DC - 1),
                        )
                    with nc.allow_low_precision(reason="bf16 matmul operands"):
                        nc.vector.tensor_copy(
                            v1_t[:, st, :].rearrange("p (h c) -> p h c", c=65)[:, :, 0:64],
                            vps[:].rearrange("p (h c) -> p h c", c=64),
                        )

                # ---- Q^T / K^T projections (per head pair, bf16)
                qt_tiles = [qk.tile([128, S], BF16, tag=f"qt{p}", name=f"qt{p}") for p in range(2)]
                kt_tiles = [qk.tile([128, S], BF16, tag=f"kt{p}", name=f"kt{p}") for p in range(2)]

                def kt_proj(pair, qc):
                    qs = slice(qc * QCW, (qc + 1) * QCW)
                    kps = pp.tile([128, QCW], F32, tag="qkv", bufs=1, name="kps")
                    for c in range(DC):
                        nc.tensor.matmul(
                            kps[:],
                            wk_t[:, c, pair * 128:(pair + 1) * 128],
                            xt_t[:, qc, c, :],
                            start=(c == 0),
                            stop=(c == DC - 1),
                        )
                        yield
                    with nc.allow_low_precision(reason="bf16 matmul operands"):
                        nc.vector.tensor_copy(kt_tiles[pair][:, qs], kps[:])
                    yield

                def qt_proj(pair, qc):
                    qs = slice(qc * QCW, (qc + 1) * QCW)
                    qps = pp.tile([128, QCW], F32, tag="qkv", bufs=1, name="qps")
                    for c in range(DC):
                        nc.tensor.matmul(
                            qps[:],
                            wq_t[:, c, pair * 128:(pair + 1) * 128],
                            xt_t[:, qc, c, :],
                            start=(c == 0),
                            stop=(c == DC - 1),
                        )
                        yield
                    with nc.allow_low_precision(reason="bf16 score operands"):
                        nc.vector.tensor_scalar_add(
                            qt_tiles[pair][:, qs], qps[:], bq_t[:, pair:pair + 1]
                        )
                    yield

                ctxt_tiles = [cp.tile([128, S], BF16, tag=f"ct{p}", name=f"ct{p}") for p in range(2)]

                def attention(pair, qc, feed=None, slots=2, feed_start_r=0, inline_v=False,
                              normalize_now=False):
                    qs = slice(qc * QCW, (qc + 1) * QCW)
                    ctx_ps = [pp.tile([65, QCW], F32, tag="ctx", name=f"ctx{_h}", bufs=2) for _h in range(2)]
                    for r in range(KT):
                        if feed is not None and r >= feed_start_r:
                            for _ in range(slots):
                                next(feed, None)
                        sreg = pp.tile([128, 2 * QCW], F32, tag="big")
                        expt = ep.tile([128, 2 * QCW], BF16, tag="exp")
                        for h in range(2):
                            nc.tensor.matmul(
                                sreg[:, h * QCW:(h + 1) * QCW],
                                kt_tiles[pair][64 * h:64 * (h + 1), r * 128:(r + 1) * 128],
                                qt_tiles[pair][64 * h:64 * (h + 1), qs],
                                start=True,
                                stop=True,
                                tile_position=(64 * h, 0),
                            )
                        with nc.allow_low_precision(reason="bf16 exp output"):
                            nc.scalar.activation(expt[:], sreg[:], AF.Exp, scale=0.125)
                        if inline_v and r <= 13:
                            v_proj(r + 2)
                        for h in range(2):
                            hh = 2 * pair + h
                            nc.tensor.matmul(
                                ctx_ps[h][:],
                                v1_t[:, r, 65 * hh:65 * hh + 65],
                                expt[:, h * QCW:(h + 1) * QCW],
                                start=(r == 0),
                                stop=(r == KT - 1),
                            )
                    if normalize_now:
                        # final call: nothing reuses the ctx banks, so
                        # normalize straight from PSUM (skips the SBUF
                        # staging copies on the tail critical path).
                        rds = []
                        for h in range(2):
                            dcp = mp.tile([1, QCW], F32, tag="dcp", name=f"dcp{h}")
                            nc.vector.tensor_copy(dcp[:], ctx_ps[h][64:65, :])
                            rd = mp.tile([1, QCW], F32, tag="rd", name=f"rd{h}")
                            nc.vector.reciprocal_approx_fast(rd[:], dcp[:])
                            rds.append(rd)
                        bcts = []
                        for h in range(2):
                            bct = mp.tile([64, QCW], F32, tag="bc", name=f"bct{h}")
                            nc.gpsimd.partition_broadcast(bct[:], rds[h][:])
                            bcts.append(bct)
                        for h in range(2):
                            with nc.allow_low_precision(reason="bf16 ctx"):
                                nc.vector.tensor_mul(
                                    ctxt_tiles[pair][64 * h:64 * (h + 1), qs],
                                    ctx_ps[h][0:64, :],
                                    bcts[h][:],
                                )
                        return iter(())
                    # evacuate PSUM now (frees ctx banks for the next call);
                    # the rest of the normalization is returned as a
                    # generator for injection into the next call's feed.
                    ctxus = []
                    for h in range(2):
                        ctxu = cu.tile([65, QCW], F32, tag="ctxu", name=f"ctxu{h}")
                        nc.vector.tensor_copy(ctxu[:], ctx_ps[h][:])
                        ctxus.append(ctxu)

                    def _norm_tail():
                        rds = []
                        for h in range(2):
                            dcp = mp.tile([1, QCW], F32, tag="dcp", name=f"dcp{h}")
                            nc.vector.tensor_copy(dcp[:], ctxus[h][64:65, :])
                            yield
                            rd = mp.tile([1, QCW], F32, tag="rd", name=f"rd{h}")
                            nc.vector.reciprocal_approx_fast(rd[:], dcp[:])
                            rds.append(rd)
                            yield
                        bcts = []
                        for h in range(2):
                            bct = mp.tile([64, QCW], F32, tag="bc", name=f"bct{h}")
                            nc.gpsimd.partition_broadcast(bct[:], rds[h][:])
                            bcts.append(bct)
                            yield
                        for h in range(2):
                            with nc.allow_low_precision(reason="bf16 ctx"):
                                nc.vector.tensor_mul(
                                    ctxt_tiles[pair][64 * h:64 * (h + 1), qs],
                                    ctxus[h][0:64, :],
                                    bcts[h][:],
                                )
                            yield

                    return _norm_tail()

                def outproj_sub(qc, sub):
                    q0 = qc * QCW + sub * 128
                    osb = op.tile([128, D], BF16, tag="osb")
                    for d2 in range(2):
                        ops = pp.tile([128, 512], F32, tag=("vo" if d2 == 0 else "qkv"), bufs=1)
                        for pair in range(2):
                            nc.tensor.matmul(
                                ops[:],
                                ctxt_tiles[pair][:, q0:q0 + 128],
                                wo_t[:, pair, d2 * 512:(d2 + 1) * 512],
                                start=(pair == 0),
                                stop=(pair == 1),
                            )
                            yield
                        with nc.allow_low_precision(reason="bf16 out"):
                            nc.vector.tensor_copy(osb[:, d2 * 512:(d2 + 1) * 512], ops[:])
                        yield
                    nc.sync.dma_start(out_d[q0:q0 + 128, :], osb[:])
                    yield

                def chain(*gens):
                    for g in gens:
                        yield from g

                def drain(g):
                    for _ in g:
                        pass

                # ---- schedule: minimal prefix, then ACT-bound attention
                # with everything else interleaved.
                drain(kt_proj(0, 0))
                drain(qt_proj(0, 0))
                for st in range(2):
                    v_proj(st)

                feed = chain(kt_proj(0, 1), kt_proj(0, 2), kt_proj(0, 3), qt_proj(0, 1))
                nt = attention(0, 0, feed, slots=3, feed_start_r=1, inline_v=True)
                drain(feed)
                feed = chain(nt, kt_proj(1, 0), kt_proj(1, 1), qt_proj(0, 2))
                nt = attention(0, 1, feed)
                drain(feed)
                feed = chain(nt, kt_proj(1, 2), kt_proj(1, 3), qt_proj(0, 3))
                nt = attention(0, 2, feed)
                drain(feed)
                feed = chain(nt, qt_proj(1, 0), qt_proj(1, 1))
                nt = attention(0, 3, feed)
                drain(feed)
                feed = chain(nt, qt_proj(1, 2))
                nt = attention(1, 0, feed)
                drain(feed)
                feed = chain(nt, qt_proj(1, 3), outproj_sub(0, 0), outproj_sub(0, 1))
                nt = attention(1, 1, feed)
                drain(feed)
                feed = chain(nt, outproj_sub(0, 2), outproj_sub(0, 3),
                             outproj_sub(1, 0), outproj_sub(1, 1))
                nt = attention(1, 2, feed)
                drain(feed)
                feed = chain(nt, outproj_sub(1, 2), outproj_sub(1, 3),
                             outproj_sub(2, 0), outproj_sub(2, 1))
                nt = attention(1, 3, feed, normalize_now=True)
                drain(feed)
                drain(nt)
                drain(outproj_sub(2, 2))
                drain(outproj_sub(2, 3))
                for sub in range(4):
                    drain(outproj_sub(3, sub))

    nc.compile()
    return nc


def _get_nc(repeat=1):
    key = (repeat,)
    if key not in _CACHE:
        _CACHE[key] = _build(repeat)
    return _CACHE[key]


def _bf16(a):
    import ml_dtypes

    return np.ascontiguousarray(np.asarray(a, np.float32)).astype(ml_dtypes.bfloat16)


def _make_in_maps(query_input, Wq, bq, Wk, Wv, Wo):
    x = np.asarray(query_input, dtype=np.float32)
    in_maps = []
    for core in range(NCORES):
        b, g = divmod(core, NCORES // B)
        cs = slice(g * HPC * HD, (g + 1) * HPC * HD)
        # xt[p, g, c, s] = x[b][g*512+s, c*128+p]
        xr = x[b].reshape(QC, QCW, DC, 128).transpose(3, 0, 2, 1)
        in_maps.append({
            "xt": _bf16(xr),
            "wq": _bf16(Wq[:, cs].reshape(DC, 128, HPC * HD).transpose(1, 0, 2)),
            "wk": _bf16(Wk[:, cs].reshape(DC, 128, HPC * HD).transpose(1, 0, 2)),
            "wv": _bf16(Wv[:, cs].reshape(DC, 128, HPC * HD).transpose(1, 0, 2)),
            "wo": _bf16(Wo[cs, :].reshape(2, 128, D).transpose(1, 0, 2)),
            "bq2": np.ascontiguousarray(np.asarray(bq, np.float32)[cs].reshape(2, 128).T),
        })
    return in_maps


def kernel(query_input, Wq, bq, Wk, bk, Wv, bv, Wo, bo):
    from concourse.bass_utils import run_bass_kernel_spmd

    Wq = np.asarray(Wq, np.float32)
    Wk = np.asarray(Wk, np.float32)
    Wv = np.asarray(Wv, np.float32)
    Wo = np.asarray(Wo, np.float32)
    bq = np.asarray(bq, np.float32)
    bv = np.asarray(bv, np.float32)
    bo = np.asarray(bo, np.float32)

    nc = _get_nc()
    in_maps = _make_in_maps(query_input, Wq, bq, Wk, Wv, Wo)
    res = run_bass_kernel_spmd(nc, in_maps, core_ids=list(range(NCORES)))

    gpc = NCORES // B  # groups per batch
    out = np.zeros((B, S, D), np.float32)
    for core in range(NCORES):
        b = core // gpc
        out[b] += res.results[core]["out_p"].astype(np.float32)
    # bv correction (exact) + bo, applied once on the full output
    out += (bv @ Wo + bo)[None, None, :]
    return out


# revision 14
# speedup vs baseline: 1.0032x; 1.0021x over previous
"""Self-contained 8-core Trainium2 Bass kernel for MultiHeadAttention.

Problem: B=2, S=2048, D=1024, H=16 heads (hd=64), f32, self-attention
(no mask), eval mode (dropout = identity).

Sharding: data-parallel over B (2) x tensor-parallel over heads (4 groups
of 4 heads) = 8 cores. Each core computes, for its batch b and its 4
heads: Q/K/V projections (column-sliced), attention, and a partial
output projection (row-sliced Wo). Host sums the 4 partials per batch
and adds the (bv @ Wo + bo) correction (bv never enters the kernel:
ctx rows sum probs to 1, so (ctx+bv) @ Wo = ctx @ Wo + bv @ Wo).

Algebraic simplifications used (exact):
  - bk dropped: softmax over k is invariant to the per-q constant Q.bk.
  - softmax computed without max subtraction (scores bounded ~|s|<10).
  - bq folded into Q^T as a per-partition bias (per-q constant cancels
    in softmax).
  - row normalization deferred past the P@V matmul (scale ctx instead
    of probs); row sums obtained free via an appended ones-column in V.

v3 design (ACT-bound pipeline):
  - All inputs bf16, host-prearranged so every DMA is contiguous per
    partition; xt arrives in 4 column-group DMAs so compute starts
    after ~1MB.
  - scores^T per head pair via two tile_position row-group matmuls
    (K=64 each) running concurrently on the PE.
  - exp on ACT is the critical engine (16.8M elems/core at 1
    elem/cycle/lane ~= 110us); everything else hides behind it.
  - PV: ctx^T[65, q] += [V_h | 1]^T @ exp^T accumulated over k tiles;
    row 64 is the softmax denominator (free).
  - normalization decoupled from PSUM: ctx+denom copied to SBUF
    (frees the bank in ~0.6us), then reciprocal_approx_fast +
    partition_broadcast + mul off the critical path.
  - All projection/out-projection matmuls interleaved into the
    ACT-bound attention r-loops via generators with deadline-aware
    ordering; feed slots run before the score matmuls of each r
    iteration so a fed op is never queued behind its consumer on the
    PE FIFO.
"""

import sys

sys.path.insert(0, "/opt/trn_rl_repo")

import numpy as np

B, S, D, H, HD = 2, 2048, 1024, 16, 64
HPC = 4  # heads per core
NCORES = 8
DC = D // 128  # 8 contraction chunks
ST = S // 128  # 16 s-tiles
QCW = 512  # q chunk width
QC = S // QCW  # 4 q chunks
KT = S // 128  # 16 k tiles

_CACHE = {}


def _build(repeat=1):
    import concourse.bass as bass  # noqa: F401
    import concourse.mybir as mybir
    import concourse.tile as tile
    from concourse import bacc
    from concourse.library_config import attn as attn_lib

    F32 = mybir.dt.float32
    BF16 = mybir.dt.bfloat16
    AF = mybir.ActivationFunctionType

    nc = bacc.Bacc("TRN2", target_bir_lowering=False, debug=False)

    # host-prearranged layouts (all contiguous per partition):
    #   xt  [128, QC, DC, 512] : xt[p, g, c, s] = x[c*128+p, g*512+s]
    #   wq/wk/wv [128, DC, 256]: w[p, c, n] = W[c*128+p, n]
    #   wo  [128, 2, 1024]     : wo[p, e, n] = Wo[e*128+p, n]
    xt_d = nc.dram_tensor("xt", [128, QC, DC, QCW], BF16, kind="ExternalInput")
    wq_d = nc.dram_tensor("wq", [128, DC, HPC * HD], BF16, kind="ExternalInput")
    wk_d = nc.dram_tensor("wk", [128, DC, HPC * HD], BF16, kind="ExternalInput")
    wv_d = nc.dram_tensor("wv", [128, DC, HPC * HD], BF16, kind="ExternalInput")
    wo_d = nc.dram_tensor("wo", [128, 2, D], BF16, kind="ExternalInput")
    bq_d = nc.dram_tensor("bq2", [128, 2], F32, kind="ExternalInput")
    out_d = nc.dram_tensor("out_p", [S, D], BF16, kind="ExternalOutput")

    with tile.TileContext(nc) as tc:
        nc.gpsimd.load_library(attn_lib)
        with (
            tc.tile_pool(name="wp", bufs=1) as wp,
            tc.tile_pool(name="xp", bufs=1) as xp,
            tc.tile_pool(name="qk", bufs=1) as qk,
            tc.tile_pool(name="vp", bufs=1) as vp,
            tc.tile_pool(name="ep", bufs=4) as ep,
            tc.tile_pool(name="cp", bufs=1) as cp,
            tc.tile_pool(name="cu", bufs=4) as cu,
            tc.tile_pool(name="mp", bufs=4) as mp,
            tc.tile_pool(name="op", bufs=2) as op,
            tc.tile_pool(name="pp", bufs=2, space="PSUM") as pp,
        ):
            # ---- loads split across both HWDGE queues (SP + Act), ordered
            # by first use: V needs wv+g0, kt needs wk+g0, qt needs wq+bq.
            xt_t = xp.tile([128, QC, DC, QCW], BF16, tag="xt")
            nc.scalar.dma_start(xt_t[:, 0, 0:4], xt_d[:, 0, 0:4])
            wq_t = wp.tile([128, DC, HPC * HD], BF16, tag="wq")
            nc.scalar.dma_start(wq_t[:], wq_d[:])
            nc.scalar.dma_start(xt_t[:, 2], xt_d[:, 2])
            wk_t = wp.tile([128, DC, HPC * HD], BF16, tag="wk")
            nc.sync.dma_start(wk_t[:], wk_d[:])
            nc.sync.dma_start(xt_t[:, 0, 4:8], xt_d[:, 0, 4:8])
            wv_t = wp.tile([128, DC, HPC * HD], BF16, tag="wv")
            nc.sync.dma_start(wv_t[:], wv_d[:])
            bq_t = wp.tile([128, 2], F32, tag="bq")
            nc.sync.dma_start(bq_t[:], bq_d[:])
            nc.sync.dma_start(xt_t[:, 1], xt_d[:, 1])
            nc.sync.dma_start(xt_t[:, 3], xt_d[:, 3])
            wo_t = wp.tile([128, 2, D], BF16, tag="wo")
            nc.sync.dma_start(wo_t[:], wo_d[:])
            ones_f = wp.tile([128, 64], BF16, tag="onesf")
            nc.vector.memset(ones_f[:], 1.0)
            # warm the ACT exp table during the DMA prefix
            warm_in = wp.tile([1, 2], F32, tag="warm_i")
            nc.vector.memset(warm_in[:], 0.0)
            warm_out = wp.tile([1, 2], F32, tag="warm_o")
            nc.scalar.activation(warm_out[:], warm_in[:], AF.Exp)
            # warm the PE (HAM un-throttle needs ~3.4us of sustained matmuls)
            wps = pp.tile([64, 64], F32, tag="vo", bufs=1, name="wps")
            for i in range(150):
                nc.tensor.matmul(wps[:], ones_f[:, 0:64], ones_f[:, 0:64],
                                 start=(i == 0), stop=(i == 149))

            import contextlib
            if repeat > 1:
                _engs = [mybir.EngineType.PE, mybir.EngineType.Activation,
                         mybir.EngineType.DVE, mybir.EngineType.SP,
                         mybir.EngineType.Pool]
                rep_ctx = tc.For_i(0, repeat, hint_engines=_engs, staggered_reset=True)
            else:
                rep_ctx = contextlib.nullcontext()
            with rep_ctx:
                # ---- V projection -> v1 [s, 4*(64+1)] with ones columns
                v1_t = vp.tile([128, ST, HPC * 65], BF16, tag="v1")
                with nc.allow_low_precision(reason="bf16 matmul operands"):
                    nc.vector.tensor_copy(
                        v1_t[:].rearrange("p s (h c) -> p s h c", c=65)[:, :, :, 64],
                        ones_f[:, 0:64].rearrange("p (s h) -> p s h", s=ST),
                    )

                def v_proj(st):
                    g, off = st // 4, (st % 4) * 128
                    vps = pp.tile([128, HPC * HD], F32, tag="vo", bufs=1, name="vps")
                    for c in range(DC):
                        nc.tensor.matmul(
                            vps[:],
                            xt_t[:, g, c, off:off + 128],
                            wv_t[:, c, :],
                            start=(c == 0),
                            stop=(c == DC - 1),
                        )
                    with nc.allow_low_precision(reason="bf16 matmul operands"):
                        nc.vector.tensor_copy(
                            v1_t[:, st, :].rearrange("p (h c) -> p h c", c=65)[:, :, 0:64],
                            vps[:].rearrange("p (h c) -> p h c", c=64),
                        )

                # ---- Q^T / K^T projections (per head pair, bf16)
                qt_tiles = [qk.tile([128, S], BF16, tag=f"qt{p}", name=f"qt{p}") for p in range(2)]
                kt_tiles = [qk.tile([128, S], BF16, tag=f"kt{p}", name=f"kt{p}") for p in range(2)]

                def kt_proj(pair, qc):
                    qs = slice(qc * QCW, (qc + 1) * QCW)
                    kps = pp.tile([128, QCW], F32, tag="qkv", bufs=1, name="kps")
                    for c in range(DC):
                        nc.tensor.matmul(
                            kps[:],
                            wk_t[:, c, pair * 128:(pair + 1) * 128],
                            xt_t[:, qc, c, :],
                            start=(c == 0),
                            stop=(c == DC - 1),
                        )
                        yield
                    with nc.allow_low_precision(reason="bf16 matmul operands"):
                        nc.vector.tensor_copy(kt_tiles[pair][:, qs], kps[:])
                    yield

                def qt_proj(pair, qc):
                    qs = slice(qc * QCW, (qc + 1) * QCW)
                    qps = pp.tile([128, QCW], F32, tag="qkv", bufs=1, name="qps")
                    for c in range(DC):
                        nc.tensor.matmul(
                            qps[:],
                            wq_t[:, c, pair * 128:(pair + 1) * 128],
                            xt_t[:, qc, c, :],
                            start=(c == 0),
                            stop=(c == DC - 1),
                        )
                        yield
                    with nc.allow_low_precision(reason="bf16 score operands"):
                        nc.vector.tensor_scalar_add(
                            qt_tiles[pair][:, qs], qps[:], bq_t[:, pair:pair + 1]
                        )
                    yield

                ctxt_tiles = [cp.tile([128, S], BF16, tag=f"ct{p}", name=f"ct{p}") for p in range(2)]

                def attention(pair, qc, feed=None, slots=2, feed_start_r=0, inline_v=False,
                              normalize_now=False):
                    qs = slice(qc * QCW, (qc + 1) * QCW)
                    ctx_ps = [pp.tile([65, QCW], F32, tag="ctx", name=f"ctx{_h}", bufs=2) for _h in range(2)]
                    for r in range(KT):
                        if feed is not None and r >= feed_start_r:
                            for _ in range(slots):
                                next(feed, None)
                        sreg = pp.tile([128, 2 * QCW], F32, tag="big")
                        expt = ep.tile([128, 2 * QCW], BF16, tag="exp")
                        for h in range(2):
                            nc.tensor.matmul(
                                sreg[:, h * QCW:(h + 1) * QCW],
                                kt_tiles[pair][64 * h:64 * (h + 1), r * 128:(r + 1) * 128],
                                qt_tiles[pair][64 * h:64 * (h + 1), qs],
                                start=True,
                                stop=True,
                                tile_position=(64 * h, 0),
                            )
                        with nc.allow_low_precision(reason="bf16 exp output"):
                            nc.scalar.activation(expt[:], sreg[:], AF.Exp, scale=0.125)
                        if inline_v and r <= 13:
                            v_proj(r + 2)
                        for h in range(2):
                            hh = 2 * pair + h
                            nc.tensor.matmul(
                                ctx_ps[h][:],
                                v1_t[:, r, 65 * hh:65 * hh + 65],
                                expt[:, h * QCW:(h + 1) * QCW],
                                start=(r == 0),
                                stop=(r == KT - 1),
                            )
                    if normalize_now:
                        # final call: nothing reuses the ctx banks, so
                        # normalize straight from PSUM (skips the SBUF
                        # staging copies on the tail critical path).
                        rds = []
                        for h in range(2):
                            dcp = mp.tile([1, QCW], F32, tag="dcp", name=f"dcp{h}")
                            nc.vector.tensor_copy(dcp[:], ctx_ps[h][64:65, :])
                            rd = mp.tile([1, QCW], F32, tag="rd", name=f"rd{h}")
                            nc.vector.reciprocal_approx_fast(rd[:], dcp[:])
                            rds.append(rd)
                        bcts = []
                        for h in range(2):
                            bct = mp.tile([64, QCW], F32, tag="bc", name=f"bct{h}")
                            nc.gpsimd.partition_broadcast(bct[:], rds[h][:])
                            bcts.append(bct)
                        for h in range(2):
                            with nc.allow_low_precision(reason="bf16 ctx"):
                                nc.vector.tensor_mul(
                                    ctxt_tiles[pair][64 * h:64 * (h + 1), qs],
                                    ctx_ps[h][0:64, :],
                                    bcts[h][:],
                                )
                        return iter(())
                    # evacuate PSUM now (frees ctx banks for the next call);
                    # the rest of the normalization is returned as a
                    # generator for injection into the next call's feed.
                    ctxus = []
                    for h in range(2):
                        ctxu = cu.tile([65, QCW], F32, tag="ctxu", name=f"ctxu{h}")
                        nc.vector.tensor_copy(ctxu[:], ctx_ps[h][:])
                        ctxus.append(ctxu)

                    def _norm_tail():
                        rds = []
                        for h in range(2):
                            dcp = mp.tile([1, QCW], F32, tag="dcp", name=f"dcp{h}")
                            nc.vector.tensor_copy(dcp[:], ctxus[h][64:65, :])
                            yield
                            rd = mp.tile([1, QCW], F32, tag="rd", name=f"rd{h}")
                            nc.vector.reciprocal_approx_fast(rd[:], dcp[:])
                            rds.append(rd)
                            yield
                        bcts = []
                        for h in range(2):
                            bct = mp.tile([64, QCW], F32, tag="bc", name=f"bct{h}")
                            nc.gpsimd.partition_broadcast(bct[:], rds[h][:])
                            bcts.append(bct)
                            yield
                        for h in range(2):
                            with nc.allow_low_precision(reason="bf16 ctx"):
                                nc.vector.tensor_mul(
                                    ctxt_tiles[pair][64 * h:64 * (h + 1), qs],
                                    ctxus[h][0:64, :],
                                    bcts[h][:],
                                )
                            yield

                    return _norm_tail()

                def outproj_sub(qc, sub):
                    q0 = qc * QCW + sub * 128
                    osb = op.tile([128, D], BF16, tag="osb")
                    for d2 in range(2):
                        ops = pp.tile([128, 512], F32, tag=("vo" if d2 == 0 else "qkv"), bufs=1)
                        for pair in range(2):
                            nc.tensor.matmul(
                                ops[:],
                                ctxt_tiles[pair][:, q0:q0 + 128],
                                wo_t[:, pair, d2 * 512:(d2 + 1) * 512],
                                start=(pair == 0),
                                stop=(pair == 1),
                            )
                            yield
                        with nc.allow_low_precision(reason="bf16 out"):
                            nc.vector.tensor_copy(osb[:, d2 * 512:(d2 + 1) * 512], ops[:])
                        yield
                    nc.sync.dma_start(out_d[q0:q0 + 128, :], osb[:])
                    yield

                def chain(*gens):
                    for g in gens:
                        yield from g

                def drain(g):
                    for _ in g:
                        pass

                # ---- schedule: minimal prefix, then ACT-bound attention
                # with everything else interleaved.
                drain(kt_proj(0, 0))
                drain(qt_proj(0, 0))
                for st in range(2):
                    v_proj(st)

                feed = chain(kt_proj(0, 1), kt_proj(0, 2), kt_proj(0, 3), qt_proj(0, 1))
                nt = attention(0, 0, feed, slots=3, feed_start_r=1, inline_v=True)
                drain(feed)
                feed = chain(nt, kt_proj(1, 0), kt_proj(1, 1), qt_proj(0, 2))
                nt = attention(0, 1, feed)
                drain(feed)
                feed = chain(nt, kt_proj(1, 2), kt_proj(1, 3), qt_proj(0, 3))
                nt = attention(0, 2, feed)
                drain(feed)
                feed = chain(nt, qt_proj(1, 0), qt_proj(1, 1))
                nt = attention(0, 3, feed)
                drain(feed)
                feed = chain(nt, qt_proj(1, 2))
                nt = attention(1, 0, feed)
                drain(feed)
                feed = chain(nt, qt_proj(1, 3), outproj_sub(0, 0), outproj_sub(0, 1))
                nt = attention(1, 1, feed)
                drain(feed)
                feed = chain(nt, outproj_sub(0, 2), outproj_sub(0, 3),
                             outproj_sub(1, 0), outproj_sub(1, 1))
                nt = attention(1, 2, feed)
                drain(feed)
                feed = chain(nt, outproj_sub(1, 2), outproj_sub(1, 3),
                             outproj_sub(2, 0), outproj_sub(2, 1))
                nt = attention(1, 3, feed, normalize_now=True)
                drain(feed)
                drain(nt)
                drain(outproj_sub(2, 2))
                drain(outproj_sub(2, 3))
                twps = pp.tile([64, 64], F32, tag="vo", bufs=1, name="twps")
                for i in range(40):
                    nc.tensor.matmul(twps[:], ones_f[:, 0:64], ones_f[:, 0:64],
                                     start=(i == 0), stop=(i == 39))
                for sub in range(4):
                    drain(outproj_sub(3, sub))

    nc.compile()
    return nc


def _get_nc(repeat=1):
    key = (repeat,)
    if key not in _CACHE:
        _CACHE[key] = _build(repeat)
    return _CACHE[key]


def _bf16(a):
    import ml_dtypes

    return np.ascontiguousarray(np.asarray(a, np.float32)).astype(ml_dtypes.bfloat16)


def _make_in_maps(query_input, Wq, bq, Wk, Wv, Wo):
    x = np.asarray(query_input, dtype=np.float32)
    in_maps = []
    for core in range(NCORES):
        b, g = divmod(core, NCORES // B)
        cs = slice(g * HPC * HD, (g + 1) * HPC * HD)
        # xt[p, g, c, s] = x[b][g*512+s, c*128+p]
        xr = x[b].reshape(QC, QCW, DC, 128).transpose(3, 0, 2, 1)
        in_maps.append({
            "xt": _bf16(xr),
            "wq": _bf16(Wq[:, cs].reshape(DC, 128, HPC * HD).transpose(1, 0, 2)),
            "wk": _bf16(Wk[:, cs].reshape(DC, 128, HPC * HD).transpose(1, 0, 2)),
            "wv": _bf16(Wv[:, cs].reshape(DC, 128, HPC * HD).transpose(1, 0, 2)),
            "wo": _bf16(Wo[cs, :].reshape(2, 128, D).transpose(1, 0, 2)),
            "bq2": np.ascontiguousarray(np.asarray(bq, np.float32)[cs].reshape(2, 128).T),
        })
    return in_maps


def kernel(query_input, Wq, bq, Wk, bk, Wv, bv, Wo, bo):
    from concourse.bass_utils import run_bass_kernel_spmd

    Wq = np.asarray(Wq, np.float32)
    Wk = np.asarray(Wk, np.float32)
    Wv = np.asarray(Wv, np.float32)
    Wo = np.asarray(Wo, np.float32)
    bq = np.asarray(bq, np.float32)
    bv = np.asarray(bv, np.float32)
    bo = np.asarray(bo, np.float32)

    nc = _get_nc()
    in_maps = _make_in_maps(query_input, Wq, bq, Wk, Wv, Wo)
    res = run_bass_kernel_spmd(nc, in_maps, core_ids=list(range(NCORES)))

    gpc = NCORES // B  # groups per batch
    out = np.zeros((B, S, D), np.float32)
    for core in range(NCORES):
        b = core // gpc
        out[b] += res.results[core]["out_p"].astype(np.float32)
    # bv correction (exact) + bo, applied once on the full output
    out += (bv @ Wo + bo)[None, None, :]
    return out
